# revision 30
# baseline (speedup 1.0000x reference)
"""Trainium2 Bass kernel for nn_DiffModel_53764400611855.

The 160000-point stream collapses algebraically to per-segment coordinate
sums u[s] (segment_sum and quat rotation are linear in the points), and the
batchnorm layers cancel every bias that is constant across the 640-segment
batch (pe_b, pfc_b, o_b1, o_b2).  What remains is:

  h1T = (W_all @ o_w1)^T @ X_all          with
  W_all rows / X_all rows:
     pfcA   (128) <->  nerfA  = sin(2pi * reduce(GA' x + bA'))   [128,640]
     pfcBs   (12) <->  nerfBs = sin(...)                          [12,640]
     pfcBi+pe_w(7)<->  xT     = noise_param^T                      [7,640]
     pe_w     (3) <->  uT     = per-seg point sums / 250           [3,640]
     2*pe_w/250(3)<->  mT     = (w*(v x u) + v x (v x u)) / |q|^2  [3,640]
     temb2   (32) <->  Bsel   = kron(I32, 1_20)                   [32,640]
  then bn+relu -> @o_w2 -> bn+relu -> @o_w3 + b3.

All matmuls run in bf16 (fp32 PSUM accumulate) except the trig-argument
matmuls, which stay fp32 for phase accuracy.  sin() uses a 3-op range
reduction (f32->i32 cast rounds to nearest on this HW) + one ACT Sin with
scale=2pi.  BatchNorm moments come from bn_stats/bn_aggr; the scale, shift,
relu and bf16 cast fuse into one ACT per tile.  Only two ACT table sets are
used (silu_and_others, sqrt_and_others).

All 8 cores run the same replicated program (no collectives); core 0's
output is returned.  Hardcodes the fixed input structure: contiguous
segments of 250 points, batch_length == 250.
"""

import numpy as np
import ml_dtypes

NCORES = 8
S, C, PPP, BO = 640, 512, 250, 32
NJ = S // 128               # seg-major blocks = 5
PI = float(np.pi)
TWO_PI = float(2.0 * np.pi)
INV2PI = float(1.0 / (2.0 * np.pi))

_CACHE = {}


def _consts():
    f = np.float32
    # nerf A block: sc-flat cols 0..127 (bands 0..9 partial), with /2pi
    # prescale and bias row (0.25 turn for cos entries)
    GAs = np.zeros((8, 128), f)
    for i in range(128):
        fb, k = i // 14, i % 14
        GAs[k % 7, i] = (2.0 ** fb) * INV2PI
        GAs[7, i] = 0.25 if k >= 7 else 0.0
    # B block: sc-flat cols 128..139 (band 9, k=2..13)
    GBs = np.zeros((8, 12), f)
    for j in range(12):
        k = 2 + j
        GBs[k % 7, j] = (2.0 ** 9) * INV2PI
        GBs[7, j] = 0.25 if k >= 7 else 0.0
    freqs = np.exp(
        -np.log(10000.0) * np.arange(256, dtype=f) / 256.0
    ).astype(f)
    fq = np.zeros((2, 256), f)
    fq[0] = freqs * INV2PI
    fq[1] = 0.25
    Bsel = np.kron(np.eye(BO, dtype=f), np.ones((1, 20), f))
    return GAs, GBs, fq, np.ascontiguousarray(Bsel)


def _build_nc():
    import concourse.mybir as mybir
    import concourse.tile as tile
    from concourse import bacc, masks

    f32, i32, bf16 = mybir.dt.float32, mybir.dt.int32, mybir.dt.bfloat16
    AF = mybir.ActivationFunctionType
    ALU = mybir.AluOpType
    AX = mybir.AxisListType

    nc = bacc.Bacc(None, num_devices=NCORES)

    def din(name, shape, dt=f32):
        return nc.dram_tensor(name, shape, dt, kind="ExternalInput")

    # consolidated input blobs (few big DMAs; see _in_maps for layouts)
    d_f32A = din("f32A", [8, 1068])
    d_f32B = din("f32B", [128, 43])
    d_ts = din("ts", [1, BO], i32)
    d_bfS = din("bfS", [1, 1024], bf16)
    d_pc = din("pc", [128, NJ * PPP * 3], bf16)
    d_Wt1 = din("Wt1", [128, 2048], bf16)
    d_Wt2 = din("Wt2", [128, 2048], bf16)
    d_W3 = din("W3", [128, 1776], bf16)
    d_W4 = din("W4", [128, 263], bf16)
    d_xTb = din("xTb", [7, S], bf16)
    d_BselA = din("BselA", [13, S], bf16)
    d_BselB = din("BselB", [19, S], bf16)
    d_out = nc.dram_tensor("outT", [7, S], bf16, kind="ExternalOutput")

    with tile.TileContext(nc) as tc:
        with (
            tc.tile_pool(name="const", bufs=1) as cp,
            tc.tile_pool(name="work", bufs=1) as wp,
            tc.tile_pool(name="ps_pre", bufs=1, space="PSUM") as pp_pre,
            tc.tile_pool(name="ps_mlp", bufs=1, space="PSUM") as pp_mlp,
            tc.tile_pool(name="ps_trp", bufs=2, space="PSUM") as pp_trp,
            tc.tile_pool(name="ps_head", bufs=4, space="PSUM") as pp_head,
        ):
            # ------------- DMAs: scalar ring = small stuff -------------
            f32A = cp.tile([8, 1068], f32, tag="f32A")
            nc.scalar.dma_start(f32A[:], d_f32A[:])
            ts_i = cp.tile([1, BO], i32, tag="ts_i")
            nc.scalar.dma_start(ts_i[:], d_ts[:])
            f32B = cp.tile([128, 43], f32, tag="f32B")
            nc.scalar.dma_start(f32B[:], d_f32B[:])
            bfS = cp.tile([1, 1024], bf16, tag="bfS")
            nc.scalar.dma_start(bfS[:], d_bfS[:])
            X1 = wp.tile([57, S], bf16, tag="X1")
            nc.gpsimd.memset(X1[:], 0.0)
            nc.scalar.dma_start(X1[12:19, :], d_xTb[:])
            nc.scalar.dma_start(X1[19:32, :], d_BselA[:])
            nc.scalar.dma_start(X1[38:57, :], d_BselB[:])
            W3 = cp.tile([128, 1776], bf16, tag="W3")
            nc.scalar.dma_start(W3[:, 1024:1776], d_W3[:, 1024:1776])
            W4 = cp.tile([128, 263], bf16, tag="W4")
            nc.scalar.dma_start(W4[:], d_W4[:])
            # ---- sync ring: big transfers, need-order, no serialization
            # (HBM streams aggregate to ~400GB/s; latency, not bandwidth,
            # is the constraint)
            pc_r = d_pc.rearrange("p (j k) -> p j k", j=NJ)
            pcb = wp.tile([128, NJ, PPP * 3], bf16, tag="pcb")
            nc.sync.dma_start(pcb[:], pc_r[:])
            pcj = [(pcb, j) for j in range(NJ)]
            Wt1 = cp.tile([128, 2048], bf16, tag="Wt1")
            nc.sync.dma_start(Wt1[:], d_Wt1[:])
            Wt2 = cp.tile([128, 2048], bf16, tag="Wt2")
            nc.sync.dma_start(Wt2[:], d_Wt2[:])
            nc.sync.dma_start(W3[:, 0:1024], d_W3[:, 0:1024])

            # views
            xTf = f32A[:, 0:640]
            GAs = f32A[:, 640:768]
            GBs = f32A[:, 768:780]
            fq = f32A[0:2, 780:1036]
            npseg = f32B[:, 0:35]
            bn1g = f32B[:, 35:37]
            bn1b = f32B[:, 37:39]
            bn2g = f32B[:, 39:40]
            bn2b = f32B[:, 40:41]
            ob3c = f32B[0:7, 41:42]
            tb1r = bfS[0:1, 0:512]
            tb2r = bfS[0:1, 512:1024]
            tw1p = [Wt1[:, 512 * k:512 * (k + 1)] for k in range(4)]
            tw2 = [Wt2[:, 512 * k:512 * (k + 1)] for k in range(4)]
            ow1 = [W3[:, 256 * k:256 * (k + 1)] for k in range(4)]
            pfcAT = W3[:, 1024:1536]
            Wall = W3[:, 1536:1764].rearrange("p (k r) -> p k r", r=57)
            pewT = W3[:, 1764:1776].rearrange("p (k r) -> p k r", r=3)
            ow2c = W4[:, 0:256].rearrange("p (k n) -> p k n", n=128)
            ow3 = W4[:, 256:263]

            ident = cp.tile([128, 128], f32, tag="ident")
            masks.make_identity(nc, ident[:])
            ones1 = cp.tile([1, BO], bf16, tag="ones1")
            nc.gpsimd.memset(ones1[:], 1.0)
            dum = cp.tile([1, 1], f32, tag="dum")
            nc.gpsimd.memset(dum[:], 1.0)
            dum2 = cp.tile([1, 1], f32, tag="dum2")
            dum3 = cp.tile([1, 1], f32, tag="dum3")
            eps128 = cp.tile([128, 1], f32, tag="eps128")
            nc.gpsimd.memset(eps128[:], 1e-5)
            nc.scalar.activation(dum2[:], dum[:], AF.Silu)

            tm2 = wp.tile([2, BO], f32, tag="tm2")
            nc.vector.tensor_copy(tm2[0:1, :], ts_i[:])
            nc.scalar.dma_start(tm2[1:2, :], d_f32A[0:1, 1036:1068])

            # q6e: 32 dummy cols | per j: u(3) m(3)
            q6e = wp.tile([128, 32 + NJ * 6], f32, tag="q6e")

            def reduce_j(j):
                t, jj = pcj[j]
                nc.vector.tensor_reduce(
                    q6e[:, 32 + 6 * j:32 + 6 * j + 3],
                    t[:, jj, :].rearrange("p (c k) -> p c k", c=3),
                    axis=AX.X, op=ALU.add,
                )

            MAGIC = float(1.5 * 2.0 ** 23)

            def frac_chain(ps_ap, P, W, tag, rr_view):
                rnd_ = wp.tile([P, W], f32, tag=f"{tag}n", name=f"{tag}n")
                nc.vector.tensor_scalar(
                    rnd_[:], ps_ap, MAGIC, -MAGIC, op0=ALU.add, op1=ALU.add
                )
                nc.vector.tensor_tensor(
                    rr_view, ps_ap, rnd_[:], op=ALU.subtract
                )

            # argt: [cos0 | sin0 | cos1 | sin1] blocks of 32
            argt = pp_trp.tile([128, 128], f32, tag="trp", name="argt")
            for r in range(2):
                fsl = slice(128 * r, 128 * (r + 1))
                nc.tensor.matmul(
                    argt[:, 64 * r:64 * r + 32], fq[:, fsl], tm2[:],
                    start=True, stop=True,
                )
                nc.tensor.matmul(
                    argt[:, 64 * r + 32:64 * r + 64], fq[0:1, fsl],
                    tm2[0:1, :], start=True, stop=True,
                )
            rrT = wp.tile([128, 128], f32, tag="rrT")
            frac_chain(argt[:], 128, 128, "at", rrT[:])
            embT = wp.tile([128, 128], bf16, tag="embT")
            nc.scalar.activation(embT[:], rrT[:], AF.Sin, scale=TWO_PI)

            reduce_j(0)
            reduce_j(1)

            # nerf A args + trig
            X0 = wp.tile([128, S], bf16, tag="X0")
            rrA = wp.tile([128, S], f32, tag="rrA")
            for h in range(2):
                sl = slice(320 * h, 320 * (h + 1))
                psA = pp_pre.tile([128, 320], f32, tag="pre", name="psA")
                nc.tensor.matmul(
                    psA[:], GAs, xTf[:, sl], start=True, stop=True
                )
                frac_chain(psA[:], 128, 320, f"nA{h}", rrA[:, sl])
            nc.scalar.activation(X0[:], rrA[:], AF.Sin, scale=TWO_PI)

            reduce_j(2)

            # nerf B args + trig
            rrB = wp.tile([12, S], f32, tag="rrB")
            for h in range(2):
                sl = slice(320 * h, 320 * (h + 1))
                psB = pp_pre.tile([128, 320], f32, tag="pre", name="psB")
                nc.tensor.matmul(
                    psB[0:12, :], GBs, xTf[:, sl], start=True, stop=True
                )
                frac_chain(psB[0:12, :], 12, 320, f"nB{h}", rrB[:, sl])
            nc.scalar.activation(X1[0:12, :], rrB[:], AF.Sin, scale=TWO_PI)

            reduce_j(3)
            reduce_j(4)

            # ------------- quaternions (comp-major packed) -------------
            npsegv = npseg.rearrange("p (j c) -> p c j", c=7)
            q6v = q6e[:, 32:32 + NJ * 6].rearrange("p (j c) -> p c j", c=6)
            sq = wp.tile([128, NJ * 4], f32, tag="sq")
            sq_v = sq[:, :].rearrange("p (j c) -> p j c", c=4)
            nc.vector.tensor_mul(
                sq_v, npseg.rearrange("p (j c) -> p j c", c=7)[:, :, 3:7],
                npseg.rearrange("p (j c) -> p j c", c=7)[:, :, 3:7],
            )
            n2 = wp.tile([128, NJ], f32, tag="n2")
            nc.vector.tensor_reduce(n2[:], sq_v, axis=AX.X, op=ALU.add)
            rn2 = wp.tile([128, NJ], f32, tag="rn2")
            nc.vector.reciprocal(rn2[:], n2[:])
            v5 = wp.tile([128, 5, NJ], f32, tag="v5")
            nc.gpsimd.tensor_copy(v5[:, 0:3, :], npsegv[:, 4:7, :])
            nc.gpsimd.tensor_copy(v5[:, 3:5, :], npsegv[:, 4:6, :])
            u5 = wp.tile([128, 5, NJ], f32, tag="u5")
            nc.gpsimd.tensor_copy(u5[:, 0:3, :], q6v[:, 0:3, :])
            nc.gpsimd.tensor_copy(u5[:, 3:5, :], q6v[:, 0:2, :])
            w3q = wp.tile([128, 3, NJ], f32, tag="w3q")
            for ci in range(3):
                nc.gpsimd.tensor_copy(w3q[:, ci, :], npsegv[:, 3, :])
            rn23 = wp.tile([128, 3, NJ], f32, tag="rn23")
            for ci in range(3):
                nc.gpsimd.tensor_copy(rn23[:, ci, :], rn2[:])
            t1 = wp.tile([128, 3, NJ], f32, tag="t1")
            t2 = wp.tile([128, 3, NJ], f32, tag="t2")
            s5 = wp.tile([128, 5, NJ], f32, tag="s5")
            nc.vector.tensor_mul(t1[:], v5[:, 1:4, :], u5[:, 2:5, :])
            nc.vector.tensor_mul(t2[:], v5[:, 2:5, :], u5[:, 1:4, :])
            nc.vector.tensor_sub(s5[:, 0:3, :], t1[:], t2[:])
            nc.gpsimd.tensor_copy(s5[:, 3:5, :], s5[:, 0:2, :])
            nc.vector.tensor_mul(t1[:], v5[:, 1:4, :], s5[:, 2:5, :])
            nc.vector.tensor_mul(t2[:], v5[:, 2:5, :], s5[:, 1:4, :])
            nc.vector.tensor_sub(t1[:], t1[:], t2[:])
            nc.vector.tensor_mul(t2[:], w3q[:], s5[:, 0:3, :])
            nc.vector.tensor_add(t1[:], t1[:], t2[:])
            nc.vector.tensor_mul(q6v[:, 3:6, :], t1[:], rn23[:])

            # umT: [128, 38] slab -> psum rows 32:38 -> X1[32:38]
            for j in range(NJ):
                trj = pp_trp.tile([128, 128], f32, tag="trp", name="trj")
                nc.tensor.transpose(
                    trj[0:38, :], q6e[:, 6 * j:6 * j + 38], ident[:]
                )
                nc.vector.tensor_copy(
                    X1[32:38, 128 * j:128 * (j + 1)], trj[32:38, :]
                )

            # ------------- W_eff part 1 + timestep MLP -------------
            psW0t = pp_pre.tile([128, 320], f32, tag="pre", name="psW0t")
            psW0 = psW0t[:, 0:256]
            for k in range(4):
                nc.tensor.matmul(
                    psW0, pfcAT[:, 128 * k:128 * (k + 1)], ow1[k],
                    start=(k == 0), stop=(k == 3),
                )
            h1p = pp_mlp.tile([32, C], f32, tag="mlp", name="h1p")
            nc.tensor.matmul(h1p[:], ones1[:], tb1r, start=True, stop=False)
            for k in range(4):
                nc.tensor.matmul(
                    h1p[:], embT[:, 32 * k:32 * (k + 1)], tw1p[k],
                    start=False, stop=(k == 3),
                )
            h1s = wp.tile([32, C], f32, tag="h1s")
            nc.scalar.activation(h1s[:], h1p[:], AF.Silu)
            nc.scalar.activation(dum3[:], h1s[0:1, 0:1], AF.Sqrt)
            h1sT = wp.tile([128, 4, 32], bf16, tag="h1sT")
            for k in range(4):
                tr = pp_trp.tile([128, 128], f32, tag="trp", name="tr1")
                nc.tensor.transpose(
                    tr[:, 0:32], h1s[:, 128 * k:128 * (k + 1)],
                    ident[0:32, 0:32]
                )
                nc.vector.tensor_copy(h1sT[:, k, :], tr[:, 0:32])
            # layer-2 transposed: tm[:, 32m:32m+32] = temb2T chunk m
            # (t_b2 dropped: constant across batch, cancelled by BN1)
            tm = pp_trp.tile([128, 128], f32, tag="trp", name="tm")
            for m in range(4):
                msl = slice(32 * m, 32 * (m + 1))
                for k in range(4):
                    nc.tensor.matmul(
                        tm[:, msl],
                        tw2[k][:, 128 * m:128 * (m + 1)], h1sT[:, k, :],
                        start=(k == 0), stop=(k == 3),
                    )
            for m in range(4):
                msl = slice(32 * m, 32 * (m + 1))
                nc.vector.tensor_copy(Wall[:, m, 19:32], tm[:, 32 * m:32 * m + 13])
                nc.vector.tensor_copy(Wall[:, m, 38:57], tm[:, 32 * m + 13:32 * m + 32])

            # ------------- W_eff part 2 -------------
            nc.vector.tensor_add(
                Wall[:, :, 12:15], Wall[:, :, 12:15], pewT
            )
            psWallt = pp_pre.tile([128, 320], f32, tag="pre", name="psWallt")
            psWall = psWallt[0:57, 0:256]
            for k in range(4):
                nc.tensor.matmul(
                    psWall, Wall[:, k, :], ow1[k],
                    start=(k == 0), stop=(k == 3),
                )
            Weff0 = wp.tile([128, 256], bf16, tag="Weff0")
            nc.scalar.activation(Weff0[:], psW0, AF.Copy)
            Weffall = wp.tile([57, 256], bf16, tag="Weffall")
            nc.scalar.activation(Weffall[:], psWall, AF.Copy)

            # ------------- h1T + BN1 -------------
            stats1 = wp.tile([128, 24], f32, tag="stats1")
            bcols1 = wp.tile([128, 4], f32, tag="bcols1")
            sc1 = wp.tile([128, 4], f32, tag="sc1")
            scale1 = wp.tile([128, 2], f32, tag="scale1")
            shift1 = wp.tile([128, 2], f32, tag="shift1")
            psts = []
            for c in range(2):
                csl = slice(128 * c, 128 * (c + 1))
                pst = []
                for h in range(2):
                    sl = slice(320 * h, 320 * (h + 1))
                    ps = pp_head.tile([128, 320], f32, tag="hd",
                                      name=f"h1t{c}{h}")
                    nc.tensor.matmul(
                        ps[:], Weff0[:, csl], X0[:, sl],
                        start=True, stop=False,
                    )
                    nc.tensor.matmul(
                        ps[:], Weffall[:, csl], X1[:, sl],
                        start=False, stop=True,
                    )
                    nc.vector.bn_stats(
                        stats1[:, 12 * c + 6 * h:12 * c + 6 * h + 6], ps[:]
                    )
                    pst.append(ps)
                psts.append(pst)
                nc.vector.bn_aggr(bcols1[:, 2 * c:2 * c + 2],
                                  stats1[:, 12 * c:12 * c + 12])
            nc.scalar.activation(
                sc1[:, 0:2], bcols1[:, 1::2], AF.Sqrt, bias=eps128[:, 0:1]
            )
            nc.vector.reciprocal(sc1[:, 2:4], sc1[:, 0:2])
            nc.vector.tensor_mul(scale1[:], sc1[:, 2:4], bn1g[:])
            nc.vector.tensor_mul(shift1[:], bcols1[:, 0::2], scale1[:])
            nc.vector.tensor_sub(shift1[:], bn1b[:], shift1[:])
            relu1 = []
            for c in range(2):
                r1 = wp.tile([128, S], bf16, tag=f"relu1{c}",
                             name=f"relu1{c}")
                relu1.append(r1)
            for h in range(2):
                sl = slice(320 * h, 320 * (h + 1))
                for c in range(2):
                    if h == 0:
                        nc.scalar.activation(
                            relu1[c][:, sl], psts[c][h][:], AF.Relu,
                            bias=shift1[:, c:c + 1], scale=scale1[:, c:c + 1],
                        )
                    else:
                        nc.vector.tensor_scalar(
                            relu1[c][:, sl], psts[c][h][:],
                            scale1[:, c:c + 1], shift1[:, c:c + 1],
                            op0=ALU.mult, op1=ALU.add,
                        )
                        nc.vector.tensor_scalar_max(
                            relu1[c][:, sl], relu1[c][:, sl], 0.0
                        )

            # ------------- h2 + BN2 -------------
            stats2 = wp.tile([128, 12], f32, tag="stats2")
            ps2t = []
            for h in range(2):
                sl = slice(320 * h, 320 * (h + 1))
                ps2 = pp_head.tile([128, 320], f32, tag="hd",
                                   name=f"h2t{h}")
                for cc in range(2):
                    nc.tensor.matmul(
                        ps2[:], ow2c[:, cc, :], relu1[cc][:, sl],
                        start=(cc == 0), stop=(cc == 1),
                    )
                nc.vector.bn_stats(stats2[:, 6 * h:6 * h + 6], ps2[:])
                ps2t.append(ps2)
            bcols2 = wp.tile([128, 4], f32, tag="bcols2")
            aggr2 = bcols2[:, 0:2]
            nc.vector.bn_aggr(aggr2, stats2[:])
            std2 = bcols2[:, 2:3]
            nc.scalar.activation(std2, aggr2[:, 1:2], AF.Sqrt,
                                 bias=eps128[:, 0:1])
            rstd2 = bcols2[:, 3:4]
            nc.vector.reciprocal(rstd2, std2)
            scale2 = wp.tile([128, 2], f32, tag="scale2")
            nc.vector.tensor_mul(scale2[:, 0:1], rstd2, bn2g[:])
            nc.vector.tensor_mul(scale2[:, 1:2], aggr2[:, 0:1],
                                 scale2[:, 0:1])
            nc.vector.tensor_sub(scale2[:, 1:2], bn2b[:], scale2[:, 1:2])
            relu2 = wp.tile([128, S], bf16, tag="relu2")
            for h in range(2):
                sl = slice(320 * h, 320 * (h + 1))
                if h == 0:
                    nc.scalar.activation(
                        relu2[:, sl], ps2t[h][:], AF.Relu,
                        bias=scale2[:, 1:2], scale=scale2[:, 0:1],
                    )
                else:
                    nc.vector.tensor_scalar(
                        relu2[:, sl], ps2t[h][:],
                        scale2[:, 0:1], scale2[:, 1:2],
                        op0=ALU.mult, op1=ALU.add,
                    )
                    nc.vector.tensor_scalar_max(
                        relu2[:, sl], relu2[:, sl], 0.0
                    )

            # ------------- out -------------
            out_sb = wp.tile([7, S], bf16, tag="out_sb")
            for h in range(2):
                sl = slice(320 * h, 320 * (h + 1))
                ps3t = pp_head.tile([128, 320], f32, tag="hd",
                                    name=f"o{h}")
                ps3 = ps3t[0:7, :]
                nc.tensor.matmul(
                    ps3, ow3, relu2[:, sl], start=True, stop=True
                )
                nc.scalar.activation(
                    out_sb[:, sl], ps3, AF.Identity, bias=ob3c
                )
                nc.scalar.dma_start(d_out[:, sl], out_sb[:, sl])

    nc.compile()
    return nc


def _in_maps(inp):
    GAs, GBs, fq, Bsel = _consts()
    f = np.float32
    bf = ml_dtypes.bfloat16

    def b(x):
        return np.ascontiguousarray(np.asarray(x, dtype=f).astype(bf))

    npar = np.ascontiguousarray(inp["noise_param"], dtype=f)
    pfc_w = np.asarray(inp["pfc_w"], dtype=f)
    pe_w = np.asarray(inp["pe_w"], dtype=f)

    f32A = np.zeros((8, 1068), f)
    f32A[0:7, 0:640] = npar.T
    f32A[7, 0:640] = 1.0
    f32A[:, 640:768] = GAs
    f32A[:, 768:780] = GBs
    f32A[0:2, 780:1036] = fq
    f32A[0, 1036:1068] = 1.0

    f32B = np.zeros((128, 43), f)
    f32B[:, 0:35] = npar.reshape(NJ, 128, 7).transpose(1, 0, 2).reshape(
        128, NJ * 7)
    f32B[:, 35:37] = np.asarray(inp["bn1_g"], f).reshape(2, 128).T
    f32B[:, 37:39] = np.asarray(inp["bn1_b"], f).reshape(2, 128).T
    f32B[:, 39:40] = np.asarray(inp["bn2_g"], f).reshape(128, 1)
    f32B[:, 40:41] = np.asarray(inp["bn2_b"], f).reshape(128, 1)
    f32B[0:7, 41] = np.asarray(inp["o_b3"], f)

    bfS = np.zeros((1, 1024), f)
    bfS[0, 0:512] = np.asarray(inp["t_b1"], f)
    bfS[0, 512:1024] = np.asarray(inp["t_b2"], f)

    tw1 = np.asarray(inp["t_w1"], dtype=f)
    perm = np.concatenate([
        np.arange(0, 128), np.arange(256, 384),
        np.arange(128, 256), np.arange(384, 512),
    ])
    tw1p = tw1[perm]
    Wt1 = np.zeros((128, 2048), f)
    Wt2 = np.zeros((128, 2048), f)
    tw2 = np.asarray(inp["t_w2"], dtype=f)
    for k in range(4):
        ch = slice(128 * k, 128 * (k + 1))
        Wt1[:, 512 * k:512 * (k + 1)] = tw1p[ch]
        Wt2[:, 512 * k:512 * (k + 1)] = tw2[ch]

    # W3: ow1(1024) | pfcAT(512) | Wall(228) | pewT(12)
    ow1 = np.asarray(inp["o_w1"], dtype=f)
    A = pfc_w[7:135]
    W3 = np.zeros((128, 1776), f)
    for k in range(4):
        ch = slice(128 * k, 128 * (k + 1))
        W3[:, 256 * k:256 * (k + 1)] = ow1[ch]
        W3[:, 1024 + 128 * k:1024 + 128 * (k + 1)] = A[:, ch].T
        base = 1536 + 57 * k
        # Wall cols: 0:12 pfcBsT | 12:19 pfcBiT | 19:32 temb2T(dev) |
        #            32:35 pe_wT/250 | 35:38 2*pe_wT/250 | 38:57 temb2T(dev)
        W3[:, base + 0:base + 12] = pfc_w[135:147, ch].T
        W3[:, base + 12:base + 19] = pfc_w[0:7, ch].T
        W3[:, base + 32:base + 35] = pe_w[:, ch].T / PPP
        W3[:, base + 35:base + 38] = pe_w[:, ch].T * (2.0 / PPP)
        W3[:, 1764 + 3 * k:1764 + 3 * (k + 1)] = pe_w[:, ch].T

    # W4: ow2c(256) | ow3(7)
    ow2 = np.asarray(inp["o_w2"], dtype=f)
    W4 = np.zeros((128, 263), f)
    for k in range(2):
        W4[:, 128 * k:128 * (k + 1)] = ow2[128 * k:128 * (k + 1)]
    W4[:, 256:263] = np.asarray(inp["o_w3"], dtype=f)

    base = {
        "f32A": f32A,
        "f32B": f32B,
        "ts": np.ascontiguousarray(
            np.asarray(inp["timesteps"]).reshape(1, BO).astype(np.int32)
        ),
        "bfS": b(bfS),
        "Wt1": b(Wt1),
        "Wt2": b(Wt2),
        "W3": b(W3),
        "W4": b(W4),
        "pc": b(
            np.asarray(inp["part_pcs"], dtype=f)
            .reshape(NJ, 128, PPP, 3).transpose(1, 0, 3, 2)
            .reshape(128, NJ * 3 * PPP)
        ),
        "xTb": b(npar.T),
        "BselA": b(Bsel[0:13]),
        "BselB": b(Bsel[13:32]),
    }
    return [dict(base) for _ in range(NCORES)]


def _ensure_axon_hooks():
    # The agent image's `antenv` lacks `axon_hooks`; bass_utils imports it
    # unconditionally when tracing under axon. Provide it (and register the
    # real NTFF hook from trn_boot) so trace=True / BASS_TRACE=1 work.
    try:
        import antenv.axon_hooks  # noqa: F401
        return
    except ImportError:
        pass
    import sys
    import types

    mod = types.ModuleType("antenv.axon_hooks")
    _hook = [None]
    mod.set_axon_ntff_profile_hook = lambda h: _hook.__setitem__(0, h)
    mod.get_axon_ntff_profile_hook = lambda: _hook[0]
    sys.modules["antenv.axon_hooks"] = mod
    try:
        import antenv

        antenv.axon_hooks = mod
    except ImportError:
        pass
    try:
        from trn_agent_boot.trn_boot import _ntff_profile_via_ctypes

        mod.set_axon_ntff_profile_hook(
            _ntff_profile_via_ctypes("/opt/axon/libaxon_pjrt.so")
        )
    except Exception:
        pass


def _run(inputs, trace=False):
    _ensure_axon_hooks()
    from concourse.bass_utils import run_bass_kernel_spmd

    if "nc" not in _CACHE:
        _CACHE["nc"] = _build_nc()
    res = run_bass_kernel_spmd(
        _CACHE["nc"], _in_maps(inputs), list(range(NCORES)), trace=trace
    )
    out = np.ascontiguousarray(
        np.asarray(res.results[0]["outT"]).astype(np.float32).T
    )
    return out, res


def kernel(**inputs):
    inp = {k: np.asarray(v) for k, v in inputs.items()}
    out, _ = _run(inp)
    return out


# revision 31
# speedup vs baseline: 1.1294x; 1.1294x over previous
"""Trainium2 Bass kernel for nn_DiffModel_53764400611855.

The 160000-point stream collapses algebraically to per-segment coordinate
sums u[s] (segment_sum and quat rotation are linear in the points), and the
batchnorm layers cancel every bias that is constant across the 640-segment
batch (pe_b, pfc_b, o_b1, o_b2).  What remains is:

  h1T = (W_all @ o_w1)^T @ X_all          with
  W_all rows / X_all rows:
     pfcA   (128) <->  nerfA  = sin(2pi * reduce(GA' x + bA'))   [128,640]
     pfcBs   (12) <->  nerfBs = sin(...)                          [12,640]
     pfcBi+pe_w(7)<->  xT     = noise_param^T                      [7,640]
     pe_w     (3) <->  uT     = per-seg point sums / 250           [3,640]
     2*pe_w/250(3)<->  mT     = (w*(v x u) + v x (v x u)) / |q|^2  [3,640]
     temb2   (32) <->  Bsel   = kron(I32, 1_20)                   [32,640]
  then bn+relu -> @o_w2 -> bn+relu -> @o_w3 + b3.

All matmuls run in bf16 (fp32 PSUM accumulate) except the trig-argument
matmuls, which stay fp32 for phase accuracy.  sin() uses a 3-op range
reduction (f32->i32 cast rounds to nearest on this HW) + one ACT Sin with
scale=2pi.  BatchNorm moments come from bn_stats/bn_aggr; the scale, shift,
relu and bf16 cast fuse into one ACT per tile.  Only two ACT table sets are
used (silu_and_others, sqrt_and_others).

All 8 cores run the same replicated program (no collectives); core 0's
output is returned.  Hardcodes the fixed input structure: contiguous
segments of 250 points, batch_length == 250.
"""

import numpy as np
import ml_dtypes

NCORES = 8
S, C, PPP, BO = 640, 512, 250, 32
NJ = S // 128               # seg-major blocks = 5
PI = float(np.pi)
TWO_PI = float(2.0 * np.pi)
INV2PI = float(1.0 / (2.0 * np.pi))

_CACHE = {}


def _consts():
    f = np.float32
    # nerf A block: sc-flat cols 0..127 (bands 0..9 partial), with /2pi
    # prescale and bias row (0.25 turn for cos entries)
    GAs = np.zeros((8, 128), f)
    for i in range(128):
        fb, k = i // 14, i % 14
        GAs[k % 7, i] = (2.0 ** fb) * INV2PI
        GAs[7, i] = 0.25 if k >= 7 else 0.0
    # B block: sc-flat cols 128..139 (band 9, k=2..13)
    GBs = np.zeros((8, 12), f)
    for j in range(12):
        k = 2 + j
        GBs[k % 7, j] = (2.0 ** 9) * INV2PI
        GBs[7, j] = 0.25 if k >= 7 else 0.0
    freqs = np.exp(
        -np.log(10000.0) * np.arange(256, dtype=f) / 256.0
    ).astype(f)
    fq = np.zeros((2, 256), f)
    fq[0] = freqs * INV2PI
    fq[1] = 0.25
    Bsel = np.kron(np.eye(BO, dtype=f), np.ones((1, 20), f))
    return GAs, GBs, fq, np.ascontiguousarray(Bsel)


def _build_nc():
    import concourse.mybir as mybir
    import concourse.tile as tile
    from concourse import bacc, masks

    f32, i32, bf16 = mybir.dt.float32, mybir.dt.int32, mybir.dt.bfloat16
    AF = mybir.ActivationFunctionType
    ALU = mybir.AluOpType
    AX = mybir.AxisListType

    nc = bacc.Bacc(None, num_devices=NCORES)

    def din(name, shape, dt=f32):
        return nc.dram_tensor(name, shape, dt, kind="ExternalInput")

    # consolidated input blobs (few big DMAs; see _in_maps for layouts)
    d_f32A = din("f32A", [8, 1068])
    d_f32B = din("f32B", [128, 43])
    d_ts = din("ts", [1, BO], i32)
    d_bfS = din("bfS", [1, 1024], bf16)
    d_pc = din("pc", [128, NJ * PPP * 3], bf16)
    d_Wt1 = din("Wt1", [128, 2048], bf16)
    d_Wt2 = din("Wt2", [128, 2048], bf16)
    d_W3 = din("W3", [128, 1776], bf16)
    d_W4 = din("W4", [128, 263], bf16)
    d_xTb = din("xTb", [7, S], bf16)
    d_BselA = din("BselA", [13, S], bf16)
    d_BselB = din("BselB", [19, S], bf16)
    d_out = nc.dram_tensor("outT", [7, S], bf16, kind="ExternalOutput")

    with tile.TileContext(nc) as tc:
        with (
            tc.tile_pool(name="const", bufs=1) as cp,
            tc.tile_pool(name="work", bufs=1) as wp,
            tc.tile_pool(name="ps_pre", bufs=2, space="PSUM") as pp_pre,
            tc.tile_pool(name="ps_trp", bufs=2, space="PSUM") as pp_trp,
            tc.tile_pool(name="ps_head", bufs=4, space="PSUM") as pp_head,
        ):
            # ------------- DMAs: scalar ring = small stuff -------------
            f32A = cp.tile([8, 1068], f32, tag="f32A")
            nc.scalar.dma_start(f32A[:], d_f32A[:])
            ts_i = cp.tile([1, BO], i32, tag="ts_i")
            nc.scalar.dma_start(ts_i[:], d_ts[:])
            f32B = cp.tile([128, 43], f32, tag="f32B")
            nc.scalar.dma_start(f32B[:], d_f32B[:])
            bfS = cp.tile([1, 1024], bf16, tag="bfS")
            nc.scalar.dma_start(bfS[:], d_bfS[:])
            X1 = wp.tile([57, S], bf16, tag="X1")
            nc.gpsimd.memset(X1[:], 0.0)
            nc.scalar.dma_start(X1[12:19, :], d_xTb[:])
            nc.scalar.dma_start(X1[19:32, :], d_BselA[:])
            nc.scalar.dma_start(X1[38:57, :], d_BselB[:])
            W3 = cp.tile([128, 1776], bf16, tag="W3")
            nc.scalar.dma_start(W3[:, 1024:1776], d_W3[:, 1024:1776])
            W4 = cp.tile([128, 263], bf16, tag="W4")
            nc.scalar.dma_start(W4[:], d_W4[:])
            # ---- sync ring: big transfers, need-order, no serialization
            # (HBM streams aggregate to ~400GB/s; latency, not bandwidth,
            # is the constraint)
            pc_r = d_pc.rearrange("p (j k) -> p j k", j=NJ)
            pcb = wp.tile([128, NJ, PPP * 3], bf16, tag="pcb")
            nc.sync.dma_start(pcb[:], pc_r[:])
            pcj = [(pcb, j) for j in range(NJ)]
            Wt1 = cp.tile([128, 2048], bf16, tag="Wt1")
            nc.sync.dma_start(Wt1[:], d_Wt1[:])
            Wt2 = cp.tile([128, 2048], bf16, tag="Wt2")
            nc.sync.dma_start(Wt2[:], d_Wt2[:])
            nc.sync.dma_start(W3[:, 0:1024], d_W3[:, 0:1024])

            # views
            xTf = f32A[:, 0:640]
            GAs = f32A[:, 640:768]
            GBs = f32A[:, 768:780]
            fq = f32A[0:2, 780:1036]
            npseg = f32B[:, 0:35]
            bn1g = f32B[:, 35:37]
            bn1b = f32B[:, 37:39]
            bn2g = f32B[:, 39:40]
            bn2b = f32B[:, 40:41]
            ob3c = f32B[0:7, 41:42]
            tb1r = bfS[0:1, 0:512]
            tb2r = bfS[0:1, 512:1024]
            tw1p = [Wt1[:, 512 * k:512 * (k + 1)] for k in range(4)]
            tw2 = [Wt2[:, 512 * k:512 * (k + 1)] for k in range(4)]
            ow1 = [W3[:, 256 * k:256 * (k + 1)] for k in range(4)]
            pfcAT = W3[:, 1024:1536]
            Wall = W3[:, 1536:1764].rearrange("p (k r) -> p k r", r=57)
            pewT = W3[:, 1764:1776].rearrange("p (k r) -> p k r", r=3)
            ow2c = W4[:, 0:256].rearrange("p (k n) -> p k n", n=128)
            ow3 = W4[:, 256:263]

            ident = cp.tile([128, 128], f32, tag="ident")
            masks.make_identity(nc, ident[:])
            ones1 = cp.tile([1, BO], bf16, tag="ones1")
            nc.gpsimd.memset(ones1[:], 1.0)
            dum = cp.tile([1, 1], f32, tag="dum")
            nc.gpsimd.memset(dum[:], 1.0)
            dum2 = cp.tile([1, 1], f32, tag="dum2")
            dum3 = cp.tile([1, 1], f32, tag="dum3")
            eps128 = cp.tile([128, 1], f32, tag="eps128")
            nc.gpsimd.memset(eps128[:], 1e-5)
            nc.scalar.activation(dum2[:], dum[:], AF.Silu)

            tm2 = wp.tile([2, BO], f32, tag="tm2")
            nc.vector.tensor_copy(tm2[0:1, :], ts_i[:])
            nc.scalar.dma_start(tm2[1:2, :], d_f32A[0:1, 1036:1068])

            # q6e: 32 dummy cols | per j: u(3) m(3)
            q6e = wp.tile([128, 32 + NJ * 6], f32, tag="q6e")

            def reduce_j(j):
                t, jj = pcj[j]
                nc.vector.tensor_reduce(
                    q6e[:, 32 + 6 * j:32 + 6 * j + 3],
                    t[:, jj, :].rearrange("p (c k) -> p c k", c=3),
                    axis=AX.X, op=ALU.add,
                )

            MAGIC = float(1.5 * 2.0 ** 23)

            def frac_chain(ps_ap, P, W, tag, rr_view):
                rnd_ = wp.tile([P, W], f32, tag=f"{tag}n", name=f"{tag}n")
                nc.vector.tensor_scalar(
                    rnd_[:], ps_ap, MAGIC, -MAGIC, op0=ALU.add, op1=ALU.add
                )
                nc.vector.tensor_tensor(
                    rr_view, ps_ap, rnd_[:], op=ALU.subtract
                )

            # argt: [cos0 | sin0 | cos1 | sin1] blocks of 32
            argt = pp_trp.tile([128, 128], f32, tag="trp", name="argt")
            for r in range(2):
                fsl = slice(128 * r, 128 * (r + 1))
                nc.tensor.matmul(
                    argt[:, 64 * r:64 * r + 32], fq[:, fsl], tm2[:],
                    start=True, stop=True,
                )
                nc.tensor.matmul(
                    argt[:, 64 * r + 32:64 * r + 64], fq[0:1, fsl],
                    tm2[0:1, :], start=True, stop=True,
                )
            rrT = wp.tile([128, 128], f32, tag="rrT")
            frac_chain(argt[:], 128, 128, "at", rrT[:])
            embT = wp.tile([128, 128], bf16, tag="embT")
            nc.scalar.activation(embT[:], rrT[:], AF.Sin, scale=TWO_PI)

            reduce_j(0)
            reduce_j(1)

            # nerf A args + trig
            X0 = wp.tile([128, S], bf16, tag="X0")
            rrA = wp.tile([128, S], f32, tag="rrA")
            for h in range(2):
                sl = slice(320 * h, 320 * (h + 1))
                psA = pp_pre.tile([128, 320], f32, tag="pre", name="psA")
                nc.tensor.matmul(
                    psA[:], GAs, xTf[:, sl], start=True, stop=True
                )
                frac_chain(psA[:], 128, 320, f"nA{h}", rrA[:, sl])
            nc.scalar.activation(X0[:], rrA[:], AF.Sin, scale=TWO_PI)

            reduce_j(2)

            # nerf B args + trig
            rrB = wp.tile([12, S], f32, tag="rrB")
            for h in range(2):
                sl = slice(320 * h, 320 * (h + 1))
                psB = pp_pre.tile([128, 320], f32, tag="pre", name="psB")
                nc.tensor.matmul(
                    psB[0:12, :], GBs, xTf[:, sl], start=True, stop=True
                )
                frac_chain(psB[0:12, :], 12, 320, f"nB{h}", rrB[:, sl])
            nc.scalar.activation(X1[0:12, :], rrB[:], AF.Sin, scale=TWO_PI)

            reduce_j(3)
            reduce_j(4)

            # ------------- quaternions (comp-major packed) -------------
            npsegv = npseg.rearrange("p (j c) -> p c j", c=7)
            q6v = q6e[:, 32:32 + NJ * 6].rearrange("p (j c) -> p c j", c=6)
            sq = wp.tile([128, NJ * 4], f32, tag="sq")
            sq_v = sq[:, :].rearrange("p (j c) -> p j c", c=4)
            nc.vector.tensor_mul(
                sq_v, npseg.rearrange("p (j c) -> p j c", c=7)[:, :, 3:7],
                npseg.rearrange("p (j c) -> p j c", c=7)[:, :, 3:7],
            )
            n2 = wp.tile([128, NJ], f32, tag="n2")
            nc.vector.tensor_reduce(n2[:], sq_v, axis=AX.X, op=ALU.add)
            rn2 = wp.tile([128, NJ], f32, tag="rn2")
            nc.vector.reciprocal(rn2[:], n2[:])
            v5 = wp.tile([128, 5, NJ], f32, tag="v5")
            nc.gpsimd.tensor_copy(v5[:, 0:3, :], npsegv[:, 4:7, :])
            nc.gpsimd.tensor_copy(v5[:, 3:5, :], npsegv[:, 4:6, :])
            u5 = wp.tile([128, 5, NJ], f32, tag="u5")
            nc.gpsimd.tensor_copy(u5[:, 0:3, :], q6v[:, 0:3, :])
            nc.gpsimd.tensor_copy(u5[:, 3:5, :], q6v[:, 0:2, :])
            w3q = wp.tile([128, 3, NJ], f32, tag="w3q")
            for ci in range(3):
                nc.gpsimd.tensor_copy(w3q[:, ci, :], npsegv[:, 3, :])
            rn23 = wp.tile([128, 3, NJ], f32, tag="rn23")
            for ci in range(3):
                nc.gpsimd.tensor_copy(rn23[:, ci, :], rn2[:])
            t1 = wp.tile([128, 3, NJ], f32, tag="t1")
            t2 = wp.tile([128, 3, NJ], f32, tag="t2")
            s5 = wp.tile([128, 5, NJ], f32, tag="s5")
            nc.vector.tensor_mul(t1[:], v5[:, 1:4, :], u5[:, 2:5, :])
            nc.vector.tensor_mul(t2[:], v5[:, 2:5, :], u5[:, 1:4, :])
            nc.vector.tensor_sub(s5[:, 0:3, :], t1[:], t2[:])
            nc.gpsimd.tensor_copy(s5[:, 3:5, :], s5[:, 0:2, :])
            nc.vector.tensor_mul(t1[:], v5[:, 1:4, :], s5[:, 2:5, :])
            nc.vector.tensor_mul(t2[:], v5[:, 2:5, :], s5[:, 1:4, :])
            nc.vector.tensor_sub(t1[:], t1[:], t2[:])
            nc.vector.tensor_mul(t2[:], w3q[:], s5[:, 0:3, :])
            nc.vector.tensor_add(t1[:], t1[:], t2[:])
            nc.vector.tensor_mul(q6v[:, 3:6, :], t1[:], rn23[:])

            # umT: [128, 38] slab -> psum rows 32:38 -> X1[32:38]
            for j in range(NJ):
                trj = pp_trp.tile([128, 128], f32, tag="trp", name="trj")
                nc.tensor.transpose(
                    trj[0:38, :], q6e[:, 6 * j:6 * j + 38], ident[:]
                )
                nc.vector.tensor_copy(
                    X1[32:38, 128 * j:128 * (j + 1)], trj[32:38, :]
                )

            # ------------- W_eff part 1 + timestep MLP -------------
            psW0t = pp_pre.tile([128, 320], f32, tag="pre", name="psW0t")
            psW0 = psW0t[:, 0:256]
            for k in range(4):
                nc.tensor.matmul(
                    psW0, pfcAT[:, 128 * k:128 * (k + 1)], ow1[k],
                    start=(k == 0), stop=(k == 3),
                )
            h1pt = pp_head.tile([128, C], f32, tag="hd", name="h1p")
            h1p = h1pt[0:32, 0:C]
            nc.tensor.matmul(h1p[:], ones1[:], tb1r, start=True, stop=False)
            for k in range(4):
                nc.tensor.matmul(
                    h1p[:], embT[:, 32 * k:32 * (k + 1)], tw1p[k],
                    start=False, stop=(k == 3),
                )
            h1s = wp.tile([32, C], f32, tag="h1s")
            nc.scalar.activation(h1s[:], h1p[:], AF.Silu)
            nc.scalar.activation(dum3[:], h1s[0:1, 0:1], AF.Sqrt)
            h1sT = wp.tile([128, 4, 32], bf16, tag="h1sT")
            for k in range(4):
                tr = pp_trp.tile([128, 128], f32, tag="trp", name="tr1")
                nc.tensor.transpose(
                    tr[:, 0:32], h1s[:, 128 * k:128 * (k + 1)],
                    ident[0:32, 0:32]
                )
                nc.vector.tensor_copy(h1sT[:, k, :], tr[:, 0:32])
            # layer-2 transposed: tm[:, 32m:32m+32] = temb2T chunk m
            # (t_b2 dropped: constant across batch, cancelled by BN1)
            tm = pp_trp.tile([128, 128], f32, tag="trp", name="tm")
            for m in range(4):
                msl = slice(32 * m, 32 * (m + 1))
                for k in range(4):
                    nc.tensor.matmul(
                        tm[:, msl],
                        tw2[k][:, 128 * m:128 * (m + 1)], h1sT[:, k, :],
                        start=(k == 0), stop=(k == 3),
                    )
            for m in range(4):
                msl = slice(32 * m, 32 * (m + 1))
                nc.vector.tensor_copy(Wall[:, m, 19:32], tm[:, 32 * m:32 * m + 13])
                nc.vector.tensor_copy(Wall[:, m, 38:57], tm[:, 32 * m + 13:32 * m + 32])

            # ------------- W_eff part 2 -------------
            nc.vector.tensor_add(
                Wall[:, :, 12:15], Wall[:, :, 12:15], pewT
            )
            psWallt = pp_pre.tile([128, 320], f32, tag="pre", name="psWallt")
            psWall = psWallt[0:57, 0:256]
            for k in range(4):
                nc.tensor.matmul(
                    psWall, Wall[:, k, :], ow1[k],
                    start=(k == 0), stop=(k == 3),
                )
            Weff0 = wp.tile([128, 256], bf16, tag="Weff0")
            nc.scalar.activation(Weff0[:], psW0, AF.Copy)
            Weffall = wp.tile([57, 256], bf16, tag="Weffall")
            nc.scalar.activation(Weffall[:], psWall, AF.Copy)

            # ------------- h1T + BN1 -------------
            stats1 = wp.tile([128, 24], f32, tag="stats1")
            bcols1 = wp.tile([128, 4], f32, tag="bcols1")
            sc1 = wp.tile([128, 4], f32, tag="sc1")
            scale1 = wp.tile([128, 2], f32, tag="scale1")
            shift1 = wp.tile([128, 2], f32, tag="shift1")
            psts = []
            for c in range(2):
                csl = slice(128 * c, 128 * (c + 1))
                pst = []
                for h in range(2):
                    sl = slice(320 * h, 320 * (h + 1))
                    pst_ = pp_head.tile([128, C], f32, tag="hd",
                                        name=f"h1t{c}{h}")
                    ps = pst_[:, 0:320]
                    nc.tensor.matmul(
                        ps[:], Weff0[:, csl], X0[:, sl],
                        start=True, stop=False,
                    )
                    nc.tensor.matmul(
                        ps[:], Weffall[:, csl], X1[:, sl],
                        start=False, stop=True,
                    )
                    nc.vector.bn_stats(
                        stats1[:, 12 * c + 6 * h:12 * c + 6 * h + 6], ps[:]
                    )
                    pst.append(ps)
                psts.append(pst)
                nc.vector.bn_aggr(bcols1[:, 2 * c:2 * c + 2],
                                  stats1[:, 12 * c:12 * c + 12])
            nc.scalar.activation(
                sc1[:, 0:2], bcols1[:, 1::2], AF.Sqrt, bias=eps128[:, 0:1]
            )
            nc.vector.reciprocal(sc1[:, 2:4], sc1[:, 0:2])
            nc.vector.tensor_mul(scale1[:], sc1[:, 2:4], bn1g[:])
            nc.vector.tensor_mul(shift1[:], bcols1[:, 0::2], scale1[:])
            nc.vector.tensor_sub(shift1[:], bn1b[:], shift1[:])
            relu1 = []
            for c in range(2):
                r1 = wp.tile([128, S], bf16, tag=f"relu1{c}",
                             name=f"relu1{c}")
                relu1.append(r1)
            for h in range(2):
                sl = slice(320 * h, 320 * (h + 1))
                for c in range(2):
                    if h == 0:
                        nc.scalar.activation(
                            relu1[c][:, sl], psts[c][h][:], AF.Relu,
                            bias=shift1[:, c:c + 1], scale=scale1[:, c:c + 1],
                        )
                    else:
                        nc.vector.tensor_scalar(
                            relu1[c][:, sl], psts[c][h][:],
                            scale1[:, c:c + 1], shift1[:, c:c + 1],
                            op0=ALU.mult, op1=ALU.add,
                        )
                        nc.vector.tensor_scalar_max(
                            relu1[c][:, sl], relu1[c][:, sl], 0.0
                        )

            # ------------- h2 + BN2 -------------
            stats2 = wp.tile([128, 12], f32, tag="stats2")
            ps2t = []
            for h in range(2):
                sl = slice(320 * h, 320 * (h + 1))
                ps2t_ = pp_head.tile([128, C], f32, tag="hd",
                                     name=f"h2t{h}")
                ps2 = ps2t_[:, 0:320]
                for cc in range(2):
                    nc.tensor.matmul(
                        ps2[:], ow2c[:, cc, :], relu1[cc][:, sl],
                        start=(cc == 0), stop=(cc == 1),
                    )
                nc.vector.bn_stats(stats2[:, 6 * h:6 * h + 6], ps2[:])
                ps2t.append(ps2)
            bcols2 = wp.tile([128, 4], f32, tag="bcols2")
            aggr2 = bcols2[:, 0:2]
            nc.vector.bn_aggr(aggr2, stats2[:])
            std2 = bcols2[:, 2:3]
            nc.scalar.activation(std2, aggr2[:, 1:2], AF.Sqrt,
                                 bias=eps128[:, 0:1])
            rstd2 = bcols2[:, 3:4]
            nc.vector.reciprocal(rstd2, std2)
            scale2 = wp.tile([128, 2], f32, tag="scale2")
            nc.vector.tensor_mul(scale2[:, 0:1], rstd2, bn2g[:])
            nc.vector.tensor_mul(scale2[:, 1:2], aggr2[:, 0:1],
                                 scale2[:, 0:1])
            nc.vector.tensor_sub(scale2[:, 1:2], bn2b[:], scale2[:, 1:2])
            relu2 = wp.tile([128, S], bf16, tag="relu2")
            for h in range(2):
                sl = slice(320 * h, 320 * (h + 1))
                if h == 0:
                    nc.scalar.activation(
                        relu2[:, sl], ps2t[h][:], AF.Relu,
                        bias=scale2[:, 1:2], scale=scale2[:, 0:1],
                    )
                else:
                    nc.vector.tensor_scalar(
                        relu2[:, sl], ps2t[h][:],
                        scale2[:, 0:1], scale2[:, 1:2],
                        op0=ALU.mult, op1=ALU.add,
                    )
                    nc.vector.tensor_scalar_max(
                        relu2[:, sl], relu2[:, sl], 0.0
                    )

            # ------------- out -------------
            out_sb = wp.tile([7, S], bf16, tag="out_sb")
            for h in range(2):
                sl = slice(320 * h, 320 * (h + 1))
                ps3t = pp_head.tile([128, C], f32, tag="hd",
                                    name=f"o{h}")
                ps3 = ps3t[0:7, 0:320]
                nc.tensor.matmul(
                    ps3, ow3, relu2[:, sl], start=True, stop=True
                )
                nc.scalar.activation(
                    out_sb[:, sl], ps3, AF.Identity, bias=ob3c
                )
                nc.scalar.dma_start(d_out[:, sl], out_sb[:, sl])

    nc.compile()
    return nc


def _in_maps(inp):
    GAs, GBs, fq, Bsel = _consts()
    f = np.float32
    bf = ml_dtypes.bfloat16

    def b(x):
        return np.ascontiguousarray(np.asarray(x, dtype=f).astype(bf))

    npar = np.ascontiguousarray(inp["noise_param"], dtype=f)
    pfc_w = np.asarray(inp["pfc_w"], dtype=f)
    pe_w = np.asarray(inp["pe_w"], dtype=f)

    f32A = np.zeros((8, 1068), f)
    f32A[0:7, 0:640] = npar.T
    f32A[7, 0:640] = 1.0
    f32A[:, 640:768] = GAs
    f32A[:, 768:780] = GBs
    f32A[0:2, 780:1036] = fq
    f32A[0, 1036:1068] = 1.0

    f32B = np.zeros((128, 43), f)
    f32B[:, 0:35] = npar.reshape(NJ, 128, 7).transpose(1, 0, 2).reshape(
        128, NJ * 7)
    f32B[:, 35:37] = np.asarray(inp["bn1_g"], f).reshape(2, 128).T
    f32B[:, 37:39] = np.asarray(inp["bn1_b"], f).reshape(2, 128).T
    f32B[:, 39:40] = np.asarray(inp["bn2_g"], f).reshape(128, 1)
    f32B[:, 40:41] = np.asarray(inp["bn2_b"], f).reshape(128, 1)
    f32B[0:7, 41] = np.asarray(inp["o_b3"], f)

    bfS = np.zeros((1, 1024), f)
    bfS[0, 0:512] = np.asarray(inp["t_b1"], f)
    bfS[0, 512:1024] = np.asarray(inp["t_b2"], f)

    tw1 = np.asarray(inp["t_w1"], dtype=f)
    perm = np.concatenate([
        np.arange(0, 128), np.arange(256, 384),
        np.arange(128, 256), np.arange(384, 512),
    ])
    tw1p = tw1[perm]
    Wt1 = np.zeros((128, 2048), f)
    Wt2 = np.zeros((128, 2048), f)
    tw2 = np.asarray(inp["t_w2"], dtype=f)
    for k in range(4):
        ch = slice(128 * k, 128 * (k + 1))
        Wt1[:, 512 * k:512 * (k + 1)] = tw1p[ch]
        Wt2[:, 512 * k:512 * (k + 1)] = tw2[ch]

    # W3: ow1(1024) | pfcAT(512) | Wall(228) | pewT(12)
    ow1 = np.asarray(inp["o_w1"], dtype=f)
    A = pfc_w[7:135]
    W3 = np.zeros((128, 1776), f)
    for k in range(4):
        ch = slice(128 * k, 128 * (k + 1))
        W3[:, 256 * k:256 * (k + 1)] = ow1[ch]
        W3[:, 1024 + 128 * k:1024 + 128 * (k + 1)] = A[:, ch].T
        base = 1536 + 57 * k
        # Wall cols: 0:12 pfcBsT | 12:19 pfcBiT | 19:32 temb2T(dev) |
        #            32:35 pe_wT/250 | 35:38 2*pe_wT/250 | 38:57 temb2T(dev)
        W3[:, base + 0:base + 12] = pfc_w[135:147, ch].T
        W3[:, base + 12:base + 19] = pfc_w[0:7, ch].T
        W3[:, base + 32:base + 35] = pe_w[:, ch].T / PPP
        W3[:, base + 35:base + 38] = pe_w[:, ch].T * (2.0 / PPP)
        W3[:, 1764 + 3 * k:1764 + 3 * (k + 1)] = pe_w[:, ch].T

    # W4: ow2c(256) | ow3(7)
    ow2 = np.asarray(inp["o_w2"], dtype=f)
    W4 = np.zeros((128, 263), f)
    for k in range(2):
        W4[:, 128 * k:128 * (k + 1)] = ow2[128 * k:128 * (k + 1)]
    W4[:, 256:263] = np.asarray(inp["o_w3"], dtype=f)

    base = {
        "f32A": f32A,
        "f32B": f32B,
        "ts": np.ascontiguousarray(
            np.asarray(inp["timesteps"]).reshape(1, BO).astype(np.int32)
        ),
        "bfS": b(bfS),
        "Wt1": b(Wt1),
        "Wt2": b(Wt2),
        "W3": b(W3),
        "W4": b(W4),
        "pc": b(
            np.asarray(inp["part_pcs"], dtype=f)
            .reshape(NJ, 128, PPP, 3).transpose(1, 0, 3, 2)
            .reshape(128, NJ * 3 * PPP)
        ),
        "xTb": b(npar.T),
        "BselA": b(Bsel[0:13]),
        "BselB": b(Bsel[13:32]),
    }
    return [dict(base) for _ in range(NCORES)]


def _ensure_axon_hooks():
    # The agent image's `antenv` lacks `axon_hooks`; bass_utils imports it
    # unconditionally when tracing under axon. Provide it (and register the
    # real NTFF hook from trn_boot) so trace=True / BASS_TRACE=1 work.
    try:
        import antenv.axon_hooks  # noqa: F401
        return
    except ImportError:
        pass
    import sys
    import types

    mod = types.ModuleType("antenv.axon_hooks")
    _hook = [None]
    mod.set_axon_ntff_profile_hook = lambda h: _hook.__setitem__(0, h)
    mod.get_axon_ntff_profile_hook = lambda: _hook[0]
    sys.modules["antenv.axon_hooks"] = mod
    try:
        import antenv

        antenv.axon_hooks = mod
    except ImportError:
        pass
    try:
        from trn_agent_boot.trn_boot import _ntff_profile_via_ctypes

        mod.set_axon_ntff_profile_hook(
            _ntff_profile_via_ctypes("/opt/axon/libaxon_pjrt.so")
        )
    except Exception:
        pass


def _run(inputs, trace=False):
    _ensure_axon_hooks()
    from concourse.bass_utils import run_bass_kernel_spmd

    if "nc" not in _CACHE:
        _CACHE["nc"] = _build_nc()
    res = run_bass_kernel_spmd(
        _CACHE["nc"], _in_maps(inputs), list(range(NCORES)), trace=trace
    )
    out = np.ascontiguousarray(
        np.asarray(res.results[0]["outT"]).astype(np.float32).T
    )
    return out, res


def kernel(**inputs):
    inp = {k: np.asarray(v) for k, v in inputs.items()}
    out, _ = _run(inp)
    return out
